# revision 1
# baseline (speedup 1.0000x reference)
"""Trainium2 Bass kernel for nn_Compute_all_u (embedding gather + batched affine dot).

Computes, for each voxel v:
    u[v, :] = coeffs[e_v, 0, :] + x_v*coeffs[e_v, 1, :] + y_v*coeffs[e_v, 2, :] + z_v*coeffs[e_v, 3, :]
where e_v = voxels_elements[v], (x,y,z) = all_voxels_centroids[v].

Sharding: data-parallel over the voxel axis across 8 NeuronCores; the
24MB coeff table stays in HBM on every core.

Gather mechanism: the TRN2 runtime's indirect DMA honors exactly ONE
dynamic row-offset per SBUF partition per instruction (one descriptor
per partition; extra offsets in the offset AP are ignored — verified on
hardware). So each gather instruction fetches 128 rows of 12 floats:
partition p <- table[idx[p, k]] for instruction k. K instructions fill a
wide [128, 12K] tile, then 6 strided DVE tensor_tensor ops compute u for
all 128*K voxels of the tile at once.

Per-core voxel layout (host-side reshape, no permutation): voxel
v = t*128*K + p*K + k <-> tile t, partition p, slot k.
"""

import numpy as np

from concourse import bacc, bass, tile, mybir
from concourse.bass_utils import run_bass_kernel_spmd

N_VOXELS = 8_000_000
N_ELEM = 500_000
N_CORES = 8
P = 128

NPC = N_VOXELS // N_CORES  # 1_000_000 voxels per core
K = 489                    # voxels per partition per tile (gathers per tile)
TILES = 16                 # tiles per core
NPC_PAD = TILES * P * K    # 1_001_472


def build_nc(n_elem: int, k: int, tiles: int, bufs: int = 3) -> bass.Bass:
    # Bacc (not raw Bass): its compile pass splits multi-sem waits into
    # event semaphores — the TRN2 ISA allows at most one wait per
    # instruction and walrus codegen rejects Tile's raw output otherwise.
    nc = bacc.Bacc("TRN2")
    f32 = mybir.dt.float32

    idx_in = nc.declare_dram_parameter("idx", [tiles, P, k], mybir.dt.int32, isOutput=False)
    cent_in = nc.declare_dram_parameter("cent", [tiles, P, 3 * k], f32, isOutput=False)
    table = nc.declare_dram_parameter("table", [n_elem, 12], f32, isOutput=False)
    out = nc.declare_dram_parameter("out", [tiles, P, 3 * k], f32, isOutput=True)

    with tile.TileContext(nc) as tc:
        with (
            tc.tile_pool(name="io", bufs=bufs) as io_pool,
            tc.tile_pool(name="tmp", bufs=2) as tmp_pool,
        ):
            for t in range(tiles):
                idx_t = io_pool.tile([P, k], mybir.dt.int32, tag="idx")
                nc.sync.dma_start(out=idx_t[:], in_=idx_in[t])

                cent_t = io_pool.tile([P, 3 * k], f32, tag="cent")
                nc.sync.dma_start(out=cent_t[:], in_=cent_in[t])

                g = io_pool.tile([P, 12 * k], f32, tag="g")
                # one indirect DMA per 128 rows: partition p <- table[idx_t[p, kk]]
                for kk in range(k):
                    nc.gpsimd.indirect_dma_start(
                        out=g[:, 12 * kk:12 * (kk + 1)],
                        out_offset=None,
                        in_=table[:],
                        in_offset=bass.IndirectOffsetOnAxis(ap=idx_t[:, kk:kk + 1], axis=0),
                    )

                # g layout per voxel slot kk: [d=0..3][j=0..2]; centroids [j=0..2]
                gr = g[:].rearrange("p (k d j) -> p k d j", d=4, j=3)
                cr = cent_t[:].rearrange("p (k j) -> p k j", j=3)

                u = io_pool.tile([P, 3 * k], f32, tag="u")
                ur = u[:].rearrange("p (k j) -> p k j", j=3)

                mul = mybir.AluOpType.mult
                add = mybir.AluOpType.add

                tmp = tmp_pool.tile([P, 3 * k], f32, tag="t")
                tr = tmp[:].rearrange("p (k j) -> p k j", j=3)

                x_b = cr[:, :, 0:1].to_broadcast([P, k, 3])
                y_b = cr[:, :, 1:2].to_broadcast([P, k, 3])
                z_b = cr[:, :, 2:3].to_broadcast([P, k, 3])

                nc.vector.tensor_tensor(out=tr, in0=x_b, in1=gr[:, :, 1, :], op=mul)
                nc.vector.tensor_tensor(out=ur, in0=gr[:, :, 0, :], in1=tr, op=add)
                nc.vector.tensor_tensor(out=tr, in0=y_b, in1=gr[:, :, 2, :], op=mul)
                nc.vector.tensor_tensor(out=ur, in0=ur, in1=tr, op=add)
                nc.vector.tensor_tensor(out=tr, in0=z_b, in1=gr[:, :, 3, :], op=mul)
                nc.vector.tensor_tensor(out=ur, in0=ur, in1=tr, op=add)

                nc.sync.dma_start(out=out[t], in_=u[:])
    nc.finalize()
    return nc


_NC_CACHE: dict = {}


def _get_nc():
    key = (N_ELEM, K, TILES)
    if key not in _NC_CACHE:
        _NC_CACHE[key] = build_nc(*key)
    return _NC_CACHE[key]


def _shard_inputs(all_coeffs, all_voxels_centroids, voxels_elements):
    table = np.ascontiguousarray(all_coeffs.reshape(N_ELEM, 12), dtype=np.float32)
    in_maps = []
    for c in range(N_CORES):
        lo, hi = c * NPC, (c + 1) * NPC
        idx = np.zeros(NPC_PAD, dtype=np.int32)
        idx[:NPC] = voxels_elements[lo:hi].astype(np.int32)
        cent = np.zeros((NPC_PAD, 3), dtype=np.float32)
        cent[:NPC] = all_voxels_centroids[lo:hi]
        in_maps.append(
            {
                "idx": idx.reshape(TILES, P, K),
                "cent": cent.reshape(TILES, P, 3 * K),
                "table": table,
            }
        )
    return in_maps


def kernel(all_coeffs, all_voxels_centroids, voxels_elements, _trace=False, **run_kwargs):
    nc = _get_nc()
    in_maps = _shard_inputs(all_coeffs, all_voxels_centroids, voxels_elements)
    res = run_bass_kernel_spmd(
        nc, in_maps, core_ids=list(range(N_CORES)), trace=_trace, **run_kwargs
    )
    outs = []
    for c in range(N_CORES):
        o = res.results[c]["out"].reshape(NPC_PAD, 3)[:NPC]
        outs.append(o)
    full = np.concatenate(outs, axis=0).astype(np.float32)
    if _trace:
        return full, res
    return full



# revision 3
# speedup vs baseline: 4.2407x; 4.2407x over previous
"""Trainium2 Bass kernel for nn_Compute_all_u (embedding gather + batched affine dot).

For each voxel v:
    u[v, :] = C[e_v, 0, :] + x_v*C[e_v, 1, :] + y_v*C[e_v, 2, :] + z_v*C[e_v, 3, :]
where e_v = voxels_elements[v], (x,y,z) = all_voxels_centroids[v].

Sharding: data-parallel over voxels across 8 NeuronCores; each core gets the
full padded coeff table in its HBM.

Gather mechanism: InstDMAGatherAnt (gpsimd.dma_gather, Q7 `mlp` ucode
library) gathers G=1024 table rows per instruction, rotated across the 4
SWDGE queues (each queue runs on its own Q7 core pair, so descriptor
generation for 4 gathers proceeds in parallel — measured ~3x faster than a
single queue). Constraints that shape the layout (all hardware-verified):
  - G > 1024 crashes the Q7 kernel (idx-read window limit), so G=1024.
  - indices are int16 -> element space is cut into NW windows of WE<=32768
    elements; each gather targets one window via the table AP base offset.
  - gathered elements must be 256B-aligned -> rows padded 48B -> 256B.

Host prep (numpy, off the HW critical path): per core, bucket voxels by
window (stable argsort), pad each window's bucket to CAP slots (pad indices
0; results discarded), build slot-ordered centroids and the slot->voxel
map, un-permute outputs. Window overflow beyond CAP (impossible for the
generated inputs, >3 sigma margin) falls back to exact host math.

Device layout: 16 gathers form a group sharing one idx-load DMA, one
centroid-load DMA, one 6-op DVE pass, and one output-store DMA (keeps SP
and DVE instruction counts ~60x lower than per-gather issue).
Slot s = ((group*16 + k)*G + i): gather k of its group, position i ->
SBUF partition i%128, chunk i//128; its int16 index sits at
idx[i%16, 64*k + i//16] (replicated across the 8 16-partition groups).
"""

import numpy as np

from concourse import bacc, bass, tile, mybir
from concourse.bass_utils import run_bass_kernel_spmd

N_VOXELS = 8_000_000
N_ELEM = 500_000
N_CORES = 8
NPC = N_VOXELS // N_CORES   # 1M voxels per core

WE = 29412                  # elements per window (< 32768 for int16 idx)
NW = 17                     # windows (17*29412 = 500004 >= 500000)
G = 1024                    # gather positions per instruction (HW limit)
TPW = 59                    # gather tiles per window
CAP = G * TPW               # 60416 voxel slots per window (max seen 59524)
GRP = 16                    # gathers per instruction group
NT = 1008                   # gather tiles per core (NW*TPW=1003 real + 5 pad)
NGRP = NT // GRP            # 63 groups
NSLOT = NT * G              # 1,032,192 slots per core
CHUNK = G // 128            # 8 chunks per partition per gather
ROW = 64                    # padded table row: 64 f32 = 256B
N_ROWS = NW * WE            # 500,004 padded table rows
NQ = 4                      # SWDGE queues


def _tile_window(t):
    return min(t // TPW, NW - 1)  # pad tiles (t >= 1003) use the last window


def build_nc(bufs: int = 3) -> bass.Bass:
    nc = bacc.Bacc("TRN2", num_swdge_queues=NQ)
    f32 = mybir.dt.float32
    i16 = mybir.dt.int16

    table = nc.declare_dram_parameter("table", [N_ROWS, ROW], f32, isOutput=False)
    idx_in = nc.declare_dram_parameter("idx", [NGRP, 128, GRP * (G // 16)], i16, isOutput=False)
    cent_in = nc.declare_dram_parameter("cent", [NGRP, 128, GRP * CHUNK * 3], f32, isOutput=False)
    out = nc.declare_dram_parameter("out", [NGRP, 128, GRP * CHUNK * 3], f32, isOutput=True)

    with tile.TileContext(nc) as tc:
        with (
            tc.tile_pool(name="io", bufs=bufs) as io_pool,
            tc.tile_pool(name="tmp", bufs=2) as tmp_pool,
        ):
            for gg in range(NGRP):
                idx_t = io_pool.tile([128, GRP * (G // 16)], i16, tag="idx")
                nc.sync.dma_start(out=idx_t[:], in_=idx_in[gg])

                cent_t = io_pool.tile([128, GRP * CHUNK * 3], f32, tag="cent")
                nc.sync.dma_start(out=cent_t[:], in_=cent_in[gg])

                g = io_pool.tile([128, GRP * CHUNK * ROW], f32, tag="g")
                for k in range(GRP):
                    t = gg * GRP + k
                    w = _tile_window(t)
                    gr_k = g[:, k * CHUNK * ROW:(k + 1) * CHUNK * ROW].rearrange(
                        "p (c r) -> p c r", r=ROW
                    )
                    nc.gpsimd.dma_gather(
                        out_ap=gr_k,
                        in_ap=table[w * WE:(w + 1) * WE],
                        idxs_ap=idx_t[:, k * (G // 16):(k + 1) * (G // 16)],
                        num_idxs=G,
                        num_idxs_reg=G,
                        elem_size=ROW,
                        queue_num=t % NQ,
                    )

                # whole-group strided views: kc = GRP*CHUNK fused chunk axis
                gr = g[:].rearrange("p (kc r) -> p kc r", r=ROW)
                cr = cent_t[:].rearrange("p (kc j) -> p kc j", j=3)
                u = io_pool.tile([128, GRP * CHUNK * 3], f32, tag="u")
                ur = u[:].rearrange("p (kc j) -> p kc j", j=3)
                tmp = tmp_pool.tile([128, GRP * CHUNK * 3], f32, tag="t")
                tr = tmp[:].rearrange("p (kc j) -> p kc j", j=3)

                mul = mybir.AluOpType.mult
                add = mybir.AluOpType.add
                KC = GRP * CHUNK

                x_b = cr[:, :, 0:1].to_broadcast([128, KC, 3])
                y_b = cr[:, :, 1:2].to_broadcast([128, KC, 3])
                z_b = cr[:, :, 2:3].to_broadcast([128, KC, 3])

                nc.vector.tensor_tensor(out=tr, in0=x_b, in1=gr[:, :, 3:6], op=mul)
                nc.vector.tensor_tensor(out=ur, in0=gr[:, :, 0:3], in1=tr, op=add)
                nc.vector.tensor_tensor(out=tr, in0=y_b, in1=gr[:, :, 6:9], op=mul)
                nc.vector.tensor_tensor(out=ur, in0=ur, in1=tr, op=add)
                nc.vector.tensor_tensor(out=tr, in0=z_b, in1=gr[:, :, 9:12], op=mul)
                nc.vector.tensor_tensor(out=ur, in0=ur, in1=tr, op=add)

                nc.sync.dma_start(out=out[gg], in_=u[:])
    nc.finalize()
    return nc


_NC_CACHE: dict = {}


def _get_nc():
    key = (G, TPW, GRP)
    if key not in _NC_CACHE:
        _NC_CACHE[key] = build_nc()
    return _NC_CACHE[key]


def _prep_core(e32, cent, table_pad):
    """Bucket one core's voxels by element window; build device arrays."""
    w = e32 // WE
    order = np.argsort(w, kind="stable")
    ws = w[order]
    counts = np.bincount(ws, minlength=NW)
    starts = np.zeros(NW, dtype=np.int64)
    starts[1:] = np.cumsum(counts)[:-1]
    rank = np.arange(NPC, dtype=np.int64) - starts[ws]
    ok = rank < CAP
    slots = ws[ok] * CAP + rank[ok]
    voxel_ids = order[ok]

    idx16 = np.zeros(NSLOT, dtype=np.int16)
    idx16[slots] = (e32[voxel_ids] - ws[ok] * WE).astype(np.int16)
    cent_s = np.zeros((NSLOT, 3), dtype=np.float32)
    cent_s[slots] = cent[voxel_ids]

    # device layouts
    # idx: tile t, pos i -> [16-block row i%16, col t*64 + i//16], replicated x8
    idx_dev = np.tile(
        idx16.reshape(NGRP, GRP * (G // 16), 16).transpose(0, 2, 1), (1, 8, 1)
    )  # [NGRP, 128, GRP*64]
    # cent: tile t, pos i -> partition i%128, fused chunk (t%GRP)*CHUNK + i//128
    cent_dev = np.ascontiguousarray(
        cent_s.reshape(NGRP, GRP, CHUNK, 128, 3).transpose(0, 3, 1, 2, 4)
    ).reshape(NGRP, 128, GRP * CHUNK * 3)

    overflow = order[~ok]  # voxel ids not placed (host fallback)
    return (
        {"table": table_pad, "idx": idx_dev, "cent": cent_dev},
        slots,
        voxel_ids,
        overflow,
    )


def kernel(all_coeffs, all_voxels_centroids, voxels_elements, _trace=False, **run_kwargs):
    nc = _get_nc()
    coeffs12 = np.asarray(all_coeffs, dtype=np.float32).reshape(N_ELEM, 12)
    table_pad = np.zeros((N_ROWS, ROW), dtype=np.float32)
    table_pad[:N_ELEM, :12] = coeffs12
    cent_full = np.asarray(all_voxels_centroids, dtype=np.float32)
    e_full = np.asarray(voxels_elements).astype(np.int64)

    in_maps, metas = [], []
    for c in range(N_CORES):
        lo, hi = c * NPC, (c + 1) * NPC
        m, slots, voxel_ids, overflow = _prep_core(
            e_full[lo:hi].astype(np.int32), cent_full[lo:hi], table_pad
        )
        in_maps.append(m)
        metas.append((slots, voxel_ids, overflow))

    res = run_bass_kernel_spmd(
        nc, in_maps, core_ids=list(range(N_CORES)), trace=_trace, **run_kwargs
    )

    full = np.empty((N_VOXELS, 3), dtype=np.float32)
    for c in range(N_CORES):
        lo, hi = c * NPC, (c + 1) * NPC
        slots, voxel_ids, overflow = metas[c]
        u_slots = (
            res.results[c]["out"]
            .reshape(NGRP, 128, GRP, CHUNK, 3)
            .transpose(0, 2, 3, 1, 4)
            .reshape(NSLOT, 3)
        )
        out_c = full[lo:hi]
        out_c[voxel_ids] = u_slots[slots]
        if overflow.size:
            e_o = e_full[lo:hi][overflow]
            cf = np.asarray(all_coeffs, dtype=np.float32)[e_o]  # [n, 4, 3]
            xyz = cent_full[lo:hi][overflow]
            out_c[overflow] = cf[:, 0] + np.einsum("nd,ndk->nk", xyz, cf[:, 1:4])
    if _trace:
        return full, res
    return full


# revision 4
# speedup vs baseline: 4.3500x; 1.0258x over previous
"""Trainium2 Bass kernel for nn_Compute_all_u (embedding gather + batched affine dot).

For each voxel v:
    u[v, :] = C[e_v, 0, :] + x_v*C[e_v, 1, :] + y_v*C[e_v, 2, :] + z_v*C[e_v, 3, :]
where e_v = voxels_elements[v], (x,y,z) = all_voxels_centroids[v].

Sharding: data-parallel over voxels across 8 NeuronCores; each core gets the
full padded coeff table in its HBM.

Gather mechanism: InstDMAGatherAnt (gpsimd.dma_gather, Q7 `mlp` ucode
library) gathers G=1024 table rows per instruction, rotated across the 4
SWDGE queues (each queue runs on its own Q7 core pair, so descriptor
generation for 4 gathers proceeds in parallel — measured ~3x faster than a
single queue). Constraints that shape the layout (all hardware-verified):
  - G > 1024 crashes the Q7 kernel (idx-read window limit), so G=1024.
  - indices are int16 -> element space is cut into NW windows of WE<=32768
    elements; each gather targets one window via the table AP base offset.
  - gathered elements must be 256B-aligned -> rows padded 48B -> 256B.

Host prep (numpy, off the HW critical path): per core, bucket voxels by
window (stable argsort), pad each window's bucket to CAP slots (pad indices
0; results discarded), build slot-ordered centroids and the slot->voxel
map, un-permute outputs. Window overflow beyond CAP (impossible for the
generated inputs, >3 sigma margin) falls back to exact host math.

Device layout: 16 gathers form a group sharing one idx-load DMA, one
centroid-load DMA, one 6-op DVE pass, and one output-store DMA (keeps SP
and DVE instruction counts ~60x lower than per-gather issue).
Slot s = ((group*16 + k)*G + i): gather k of its group, position i ->
SBUF partition i%128, chunk i//128; its int16 index sits at
idx[i%16, 64*k + i//16] (replicated across the 8 16-partition groups).
"""

import numpy as np

from concourse import bacc, bass, tile, mybir
from concourse.bass_utils import run_bass_kernel_spmd

N_VOXELS = 8_000_000
N_ELEM = 500_000
N_CORES = 8
NPC = N_VOXELS // N_CORES   # 1M voxels per core

WE = 29412                  # elements per window (< 32768 for int16 idx)
NW = 17                     # windows (17*29412 = 500004 >= 500000)
G = 1024                    # gather positions per instruction (HW limit)
TPW = 59                    # gather tiles per window
CAP = G * TPW               # 60416 voxel slots per window (max seen 59524)
GRP = 16                    # gathers per instruction group
NT = 1008                   # gather tiles per core (NW*TPW=1003 real + 5 pad)
NGRP = NT // GRP            # 63 groups
NSLOT = NT * G              # 1,032,192 slots per core
CHUNK = G // 128            # 8 chunks per partition per gather
ROW = 64                    # padded table row: 64 f32 = 256B
N_ROWS = NW * WE            # 500,004 padded table rows
NQ = 4                      # SWDGE queues


def _tile_window(t):
    return min(t // TPW, NW - 1)  # pad tiles (t >= 1003) use the last window


def build_nc(bufs: int = 4) -> bass.Bass:
    nc = bacc.Bacc("TRN2", num_swdge_queues=NQ)
    f32 = mybir.dt.float32
    i16 = mybir.dt.int16

    table = nc.declare_dram_parameter("table", [N_ROWS, ROW], f32, isOutput=False)
    idx_in = nc.declare_dram_parameter("idx", [NGRP, 128, GRP * (G // 16)], i16, isOutput=False)
    cent_in = nc.declare_dram_parameter("cent", [NGRP, 128, GRP * CHUNK * 3], f32, isOutput=False)
    out = nc.declare_dram_parameter("out", [NGRP, 128, GRP * CHUNK * 3], f32, isOutput=True)

    with tile.TileContext(nc) as tc:
        with (
            tc.tile_pool(name="io", bufs=bufs) as io_pool,
            tc.tile_pool(name="tmp", bufs=2) as tmp_pool,
        ):
            for gg in range(NGRP):
                idx_t = io_pool.tile([128, GRP * (G // 16)], i16, tag="idx")
                nc.sync.dma_start(out=idx_t[:], in_=idx_in[gg])

                cent_t = io_pool.tile([128, GRP * CHUNK * 3], f32, tag="cent")
                nc.sync.dma_start(out=cent_t[:], in_=cent_in[gg])

                g = io_pool.tile([128, GRP * CHUNK * ROW], f32, tag="g")
                for k in range(GRP):
                    t = gg * GRP + k
                    w = _tile_window(t)
                    gr_k = g[:, k * CHUNK * ROW:(k + 1) * CHUNK * ROW].rearrange(
                        "p (c r) -> p c r", r=ROW
                    )
                    nc.gpsimd.dma_gather(
                        out_ap=gr_k,
                        in_ap=table[w * WE:(w + 1) * WE],
                        idxs_ap=idx_t[:, k * (G // 16):(k + 1) * (G // 16)],
                        num_idxs=G,
                        num_idxs_reg=G,
                        elem_size=ROW,
                        queue_num=t % NQ,
                    )

                # whole-group strided views: kc = GRP*CHUNK fused chunk axis
                gr = g[:].rearrange("p (kc r) -> p kc r", r=ROW)
                cr = cent_t[:].rearrange("p (kc j) -> p kc j", j=3)
                u = io_pool.tile([128, GRP * CHUNK * 3], f32, tag="u")
                ur = u[:].rearrange("p (kc j) -> p kc j", j=3)
                tmp = tmp_pool.tile([128, GRP * CHUNK * 3], f32, tag="t")
                tr = tmp[:].rearrange("p (kc j) -> p kc j", j=3)

                mul = mybir.AluOpType.mult
                add = mybir.AluOpType.add
                KC = GRP * CHUNK

                x_b = cr[:, :, 0:1].to_broadcast([128, KC, 3])
                y_b = cr[:, :, 1:2].to_broadcast([128, KC, 3])
                z_b = cr[:, :, 2:3].to_broadcast([128, KC, 3])

                nc.vector.tensor_tensor(out=tr, in0=x_b, in1=gr[:, :, 3:6], op=mul)
                nc.vector.tensor_tensor(out=ur, in0=gr[:, :, 0:3], in1=tr, op=add)
                nc.vector.tensor_tensor(out=tr, in0=y_b, in1=gr[:, :, 6:9], op=mul)
                nc.vector.tensor_tensor(out=ur, in0=ur, in1=tr, op=add)
                nc.vector.tensor_tensor(out=tr, in0=z_b, in1=gr[:, :, 9:12], op=mul)
                nc.vector.tensor_tensor(out=ur, in0=ur, in1=tr, op=add)

                nc.sync.dma_start(out=out[gg], in_=u[:])
    nc.finalize()
    return nc


_NC_CACHE: dict = {}


def _get_nc():
    key = (G, TPW, GRP)
    if key not in _NC_CACHE:
        _NC_CACHE[key] = build_nc()
    return _NC_CACHE[key]


def _prep_core(e32, cent, table_pad):
    """Bucket one core's voxels by element window; build device arrays."""
    w = e32 // WE
    order = np.argsort(w, kind="stable")
    ws = w[order]
    counts = np.bincount(ws, minlength=NW)
    starts = np.zeros(NW, dtype=np.int64)
    starts[1:] = np.cumsum(counts)[:-1]
    rank = np.arange(NPC, dtype=np.int64) - starts[ws]
    ok = rank < CAP
    slots = ws[ok] * CAP + rank[ok]
    voxel_ids = order[ok]

    idx16 = np.zeros(NSLOT, dtype=np.int16)
    idx16[slots] = (e32[voxel_ids] - ws[ok] * WE).astype(np.int16)
    cent_s = np.zeros((NSLOT, 3), dtype=np.float32)
    cent_s[slots] = cent[voxel_ids]

    # device layouts
    # idx: tile t, pos i -> [16-block row i%16, col t*64 + i//16], replicated x8
    idx_dev = np.tile(
        idx16.reshape(NGRP, GRP * (G // 16), 16).transpose(0, 2, 1), (1, 8, 1)
    )  # [NGRP, 128, GRP*64]
    # cent: tile t, pos i -> partition i%128, fused chunk (t%GRP)*CHUNK + i//128
    cent_dev = np.ascontiguousarray(
        cent_s.reshape(NGRP, GRP, CHUNK, 128, 3).transpose(0, 3, 1, 2, 4)
    ).reshape(NGRP, 128, GRP * CHUNK * 3)

    overflow = order[~ok]  # voxel ids not placed (host fallback)
    return (
        {"table": table_pad, "idx": idx_dev, "cent": cent_dev},
        slots,
        voxel_ids,
        overflow,
    )


def kernel(all_coeffs, all_voxels_centroids, voxels_elements, _trace=False, **run_kwargs):
    nc = _get_nc()
    coeffs12 = np.asarray(all_coeffs, dtype=np.float32).reshape(N_ELEM, 12)
    table_pad = np.zeros((N_ROWS, ROW), dtype=np.float32)
    table_pad[:N_ELEM, :12] = coeffs12
    cent_full = np.asarray(all_voxels_centroids, dtype=np.float32)
    e_full = np.asarray(voxels_elements).astype(np.int64)

    in_maps, metas = [], []
    for c in range(N_CORES):
        lo, hi = c * NPC, (c + 1) * NPC
        m, slots, voxel_ids, overflow = _prep_core(
            e_full[lo:hi].astype(np.int32), cent_full[lo:hi], table_pad
        )
        in_maps.append(m)
        metas.append((slots, voxel_ids, overflow))

    res = run_bass_kernel_spmd(
        nc, in_maps, core_ids=list(range(N_CORES)), trace=_trace, **run_kwargs
    )

    full = np.empty((N_VOXELS, 3), dtype=np.float32)
    for c in range(N_CORES):
        lo, hi = c * NPC, (c + 1) * NPC
        slots, voxel_ids, overflow = metas[c]
        u_slots = (
            res.results[c]["out"]
            .reshape(NGRP, 128, GRP, CHUNK, 3)
            .transpose(0, 2, 3, 1, 4)
            .reshape(NSLOT, 3)
        )
        out_c = full[lo:hi]
        out_c[voxel_ids] = u_slots[slots]
        if overflow.size:
            e_o = e_full[lo:hi][overflow]
            cf = np.asarray(all_coeffs, dtype=np.float32)[e_o]  # [n, 4, 3]
            xyz = cent_full[lo:hi][overflow]
            out_c[overflow] = cf[:, 0] + np.einsum("nd,ndk->nk", xyz, cf[:, 1:4])
    if _trace:
        return full, res
    return full
